# revision 2
# baseline (speedup 1.0000x reference)
"""Trainium2 Bass kernel for the 2-layer-LSTM tactile rollout (nn_ACTP).

Strategy: data-parallel over batch (B=512 -> 64 rows/core on 8 cores).
All weights stay SBUF-resident (f32r, DMA'd directly). LSTM matmuls run
batch-major (activations stationary, weights moving, float32r at N=512
-> full PE rate); FC1/FC2 run feature-major (weights stationary) so the
rollout feedback path needs no per-step transposes. Gate order is
reshuffled to [i f o g] so one sigmoid covers [64,1536] and one tanh
covers [64,512]. Gate biases ride a ones-row in the stationary operand;
FC biases use the ACT per-partition bias port. h1/h2 are re-transposed
each step via PE-transpose (4x [64,128] each). mae partials exit as a
[64,1] per-core column, reduced on host.
"""
import sys
import types

sys.path.insert(0, "/opt/trn_rl_repo")

import numpy as np

# ---- NTFF profile hook (missing antenv.axon_hooks in this image) ----
try:
    import antenv
    if "antenv.axon_hooks" not in sys.modules:
        _mod = types.ModuleType("antenv.axon_hooks")
        _hooks = [None]
        _mod.set_axon_ntff_profile_hook = lambda h: _hooks.__setitem__(0, h)
        _mod.get_axon_ntff_profile_hook = lambda: _hooks[0]
        sys.modules["antenv.axon_hooks"] = _mod
        antenv.axon_hooks = _mod
        from trn_agent_boot.trn_boot import _ntff_profile_via_ctypes
        _mod.set_axon_ntff_profile_hook(
            _ntff_profile_via_ctypes("/opt/axon/libaxon_pjrt.so")
        )
except Exception:
    pass

import concourse.bass as bass
import concourse.mybir as mybir
import concourse.tile as tile
from concourse import bacc
from concourse.bass_utils import run_bass_kernel_spmd
from concourse.masks import make_identity

F32 = mybir.dt.float32
F32R = mybir.dt.float32r

T, B, D, A, H = 101, 512, 64, 12, 512
CONTEXT = 10
NCORES = 8
BL = B // NCORES          # 64 batch rows per core
NSTEPS_FULL = T - 1       # 100
G4 = 4 * H                # 2048


def build(n_steps=NSTEPS_FULL):
    nc = bacc.Bacc("TRN2", target_bir_lowering=False, debug=False,
                   num_devices=NCORES)

    dp = lambda name, shape, dt=F32: nc.dram_tensor(
        name, shape, dt, kind="ExternalInput").ap()

    wih1_d = dp("wih1", [65, G4])
    whh1_d = dp("whh1", [4, 128, G4])
    wih2h_d = dp("wih2h", [4, 128, G4])
    wih2t_d = dp("wih2t", [97, G4])
    whh2_d = dp("whh2", [4, 128, G4])
    w1h_d = dp("w1h", [4, 128, H])
    w1t_d = dp("w1t", [64, H])
    b1_d = dp("b1", [128, 4])
    w2_d = dp("w2", [4, 128, D])
    b2_d = dp("b2", [64, 1])
    tacT_d = dp("tacT", [65, CONTEXT, BL])
    tiledT_d = dp("tiledT", [NSTEPS_FULL, 97, BL])
    targT_d = dp("targT", [NSTEPS_FULL, D, BL])

    n_out = max(n_steps - (CONTEXT - 1), 0)
    outs_d = nc.dram_tensor("outs", [max(n_out, 1), D, BL], F32,
                            kind="ExternalOutput").ap()
    mae_d = nc.dram_tensor("mae", [D, 1], F32, kind="ExternalOutput").ap()

    with tile.TileContext(nc) as tc:
        _build_body(nc, tc, n_steps, wih1_d, whh1_d, wih2h_d, wih2t_d,
                    whh2_d, w1h_d, w1t_d, b1_d, w2_d, b2_d, tacT_d,
                    tiledT_d, targT_d, outs_d, mae_d)
    nc.finalize()
    return nc


def _build_body(nc, tc, n_steps, wih1_d, whh1_d, wih2h_d, wih2t_d, whh2_d,
                w1h_d, w1t_d, b1_d, w2_d, b2_d, tacT_d, tiledT_d, targT_d,
                outs_d, mae_d):
    from contextlib import ExitStack
    ctx = ExitStack()
    wp = ctx.enter_context(tc.tile_pool(name="wp", bufs=1))
    st = ctx.enter_context(tc.tile_pool(name="st", bufs=1))
    wk = ctx.enter_context(tc.tile_pool(name="wk", bufs=2))
    sm = ctx.enter_context(tc.tile_pool(name="sm", bufs=3))
    gps = ctx.enter_context(tc.tile_pool(name="gps", bufs=1, space="PSUM"))
    tps = ctx.enter_context(tc.tile_pool(name="tps", bufs=2, space="PSUM"))
    fps = ctx.enter_context(tc.tile_pool(name="fps", bufs=2, space="PSUM"))

    r = lambda ap: ap.bitcast(F32R)

    # ---- resident weights (f32r via direct DMA) ----
    wih1 = wp.tile([65, G4], F32R)
    nc.sync.dma_start(out=wih1, in_=r(wih1_d))
    whh1 = wp.tile([128, 4, G4], F32R)
    wih2h = wp.tile([128, 4, G4], F32R)
    whh2 = wp.tile([128, 4, G4], F32R)
    w1h = wp.tile([128, 4, H], F32R)
    w2 = wp.tile([128, 4, D], F32R)
    for k in range(4):
        nc.sync.dma_start(out=whh1[:, k, :], in_=r(whh1_d[k]))
        nc.sync.dma_start(out=wih2h[:, k, :], in_=r(wih2h_d[k]))
        nc.sync.dma_start(out=whh2[:, k, :], in_=r(whh2_d[k]))
        nc.sync.dma_start(out=w1h[:, k, :], in_=r(w1h_d[k]))
        nc.sync.dma_start(out=w2[:, k, :], in_=r(w2_d[k]))
    wih2t = wp.tile([97, G4], F32R)
    nc.sync.dma_start(out=wih2t, in_=r(wih2t_d))
    w1t = wp.tile([64, H], F32R)
    nc.sync.dma_start(out=w1t, in_=r(w1t_d))
    b1 = wp.tile([128, 4], F32)
    nc.sync.dma_start(out=b1, in_=b1_d)
    b2 = wp.tile([64, 1], F32)
    nc.sync.dma_start(out=b2, in_=b2_d)
    tacT = wp.tile([65, CONTEXT, BL], F32R)
    nc.sync.dma_start(out=tacT, in_=r(tacT_d))
    ident = wp.tile([128, 128], F32)
    make_identity(nc, ident[:])

    # ---- persistent state ----
    c1 = st.tile([BL, H], F32)
    c2 = st.tile([BL, H], F32)
    h1T = st.tile([128, 4, BL], F32R)
    h2T = st.tile([128, 4, BL], F32R)
    inp1T = st.tile([65, BL], F32R)
    out3T = st.tile([128, 4, BL], F32R)
    acc = st.tile([D, 1], F32)
    nc.vector.memset(inp1T[64:65, :].bitcast(F32), 1.0)

    SIG = mybir.ActivationFunctionType.Sigmoid
    TANH = mybir.ActivationFunctionType.Tanh

    def lstm_cell(gps_tile, cstate, h_out, first):
        """gates psum [BL, 2048] ([i f o] | [g]) -> h_out [BL,H], cstate."""
        sig = wk.tile([BL, 3 * H], F32, tag="sig")
        nc.scalar.activation(sig, gps_tile[:, 0:3 * H], SIG)
        tg = wk.tile([BL, H], F32, tag="tg")
        nc.scalar.activation(tg, gps_tile[:, 3 * H:4 * H], TANH)
        if first:
            nc.vector.tensor_mul(cstate[:], sig[:, 0:H], tg[:])
        else:
            t1 = wk.tile([BL, H], F32, tag="t1")
            nc.vector.tensor_mul(t1, sig[:, H:2 * H], cstate[:])
            t2 = wk.tile([BL, H], F32, tag="t2")
            nc.vector.tensor_mul(t2, sig[:, 0:H], tg[:])
            nc.vector.tensor_add(cstate[:], t1[:], t2[:])
        tcl = wk.tile([BL, H], F32, tag="tcl")
        nc.scalar.activation(tcl, cstate[:], TANH)
        nc.vector.tensor_mul(h_out[:], sig[:, 2 * H:3 * H], tcl[:])

    def transpose_to(hT, h_bm):
        for k in range(4):
            tp = tps.tile([128, BL], F32, tag="tp")
            nc.tensor.transpose(tp, h_bm[:, 128 * k:128 * (k + 1)],
                                ident[0:BL, 0:BL])
            nc.vector.tensor_copy(hT[:, k, :], tp)

    g1_ps = None          # L1 gates psum carried across iterations
    for t in range(n_steps):
        teacher = t < CONTEXT
        inp_stat = tacT[:, t, :] if teacher else inp1T[:]
        inp_rhs = tacT[0:64, t, :] if teacher else inp1T[0:64, :]

        # (1) L1 x-part (+bias row) closes the gate groups
        if t == 0:
            g1_ps = gps.tile([BL, G4], F32, tag="gates")
        for n in range(4):
            nc.tensor.matmul(g1_ps[:, 512 * n:512 * (n + 1)], inp_stat,
                             wih1[:, 512 * n:512 * (n + 1)],
                             start=(t == 0), stop=True)

        # (2) L1 cell
        h1 = wk.tile([BL, H], F32, tag="h1")
        lstm_cell(g1_ps, c1, h1, first=(t == 0))

        # (3) L2 h-part (concurrent with L1 cell on ACT/DVE)
        g2_ps = gps.tile([BL, G4], F32, tag="gates")
        if t > 0:
            for n in range(4):
                for k in range(4):
                    nc.tensor.matmul(g2_ps[:, 512 * n:512 * (n + 1)],
                                     h2T[:, k, :],
                                     whh2[:, k, 512 * n:512 * (n + 1)],
                                     start=(k == 0), stop=False)

        # (4) h1 -> h1T
        transpose_to(h1T, h1)

        # (5) L2 x2-part: h1 contraction + tiled tail (bias row)
        tiledT = sm.tile([97, BL], F32R, tag="tiled")
        nc.sync.dma_start(out=tiledT, in_=r(tiledT_d[t]))
        for n in range(4):
            for k in range(4):
                nc.tensor.matmul(g2_ps[:, 512 * n:512 * (n + 1)],
                                 h1T[:, k, :],
                                 wih2h[:, k, 512 * n:512 * (n + 1)],
                                 start=(t == 0 and k == 0), stop=False)
            nc.tensor.matmul(g2_ps[:, 512 * n:512 * (n + 1)], tiledT[:],
                             wih2t[:, 512 * n:512 * (n + 1)],
                             start=False, stop=True)

        # (6) L2 cell
        h2 = wk.tile([BL, H], F32, tag="h2")
        lstm_cell(g2_ps, c2, h2, first=(t == 0))

        # (7) next step's L1 h-part opens the next gate groups early
        if t + 1 < n_steps:
            g1_ps = gps.tile([BL, G4], F32, tag="gates")
            for n in range(4):
                for k in range(4):
                    nc.tensor.matmul(g1_ps[:, 512 * n:512 * (n + 1)],
                                     h1T[:, k, :],
                                     whh1[:, k, 512 * n:512 * (n + 1)],
                                     start=(k == 0), stop=False)

        # (8) h2 -> h2T
        transpose_to(h2T, h2)

        # (9) FC1 (feature-major): out3T = tanh(W1 @ x3T + b1)
        for m in range(4):
            fc = fps.tile([128, BL], F32, tag="fc")
            for k in range(4):
                nc.tensor.matmul(fc, w1h[:, k, 128 * m:128 * (m + 1)],
                                 h2T[:, k, :], start=(k == 0), stop=False)
            nc.tensor.matmul(fc, w1t[:, 128 * m:128 * (m + 1)], inp_rhs,
                             start=False, stop=True)
            nc.scalar.activation(out3T[:, m, :], fc, TANH,
                                 bias=b1[:, m:m + 1])

        # (10) FC2: out4T = tanh(W2 @ out3T + b2) -> inp1T rows 0:64
        fc2 = fps.tile([D, BL], F32, tag="fc")
        for m in range(4):
            nc.tensor.matmul(fc2, w2[:, m, :], out3T[:, m, :],
                             start=(m == 0), stop=(m == 3))
        nc.scalar.activation(inp1T[0:D, :], fc2, TANH, bias=b2[:, 0:1])

        if t >= CONTEXT - 1:
            nc.gpsimd.dma_start(out=outs_d[t - (CONTEXT - 1)],
                                in_=inp1T[0:D, :].bitcast(F32))

        # (11) mae
        targT = sm.tile([D, BL], F32, tag="targ")
        nc.sync.dma_start(out=targT, in_=targT_d[t])
        diff = wk.tile([D, BL], F32, tag="diff")
        nc.vector.tensor_sub(diff, inp1T[0:D, :].bitcast(F32), targT[:])
        if t == 0:
            nc.vector.tensor_reduce(acc[:], diff[:], mybir.AxisListType.X,
                                    mybir.AluOpType.add,
                                    apply_absolute_value=True)
        else:
            red = wk.tile([D, 1], F32, tag="red")
            nc.vector.tensor_reduce(red, diff[:], mybir.AxisListType.X,
                                    mybir.AluOpType.add,
                                    apply_absolute_value=True)
            nc.vector.tensor_add(acc[:], acc[:], red[:])

    nc.gpsimd.dma_start(out=mae_d, in_=acc[:])
    ctx.close()


# ---------------- host side ----------------

_GPERM = np.concatenate([np.arange(0, 512), np.arange(512, 1024),
                         np.arange(1536, 2048), np.arange(1024, 1536)])


def prep_inputs(tactiles, actions, Wih1, Whh1, bih1, bhh1, Wih2, Whh2,
                bih2, bhh2, W1, b1, W2, b2):
    f = np.float32
    shared = {}
    wih1 = np.empty((65, G4), f)
    wih1[0:64] = Wih1.T[:, _GPERM]
    wih1[64] = (bih1 + bhh1)[_GPERM]
    shared["wih1"] = wih1
    shared["whh1"] = np.ascontiguousarray(
        Whh1.T[:, _GPERM].reshape(4, 128, G4))
    shared["wih2h"] = np.ascontiguousarray(
        Wih2.T[0:512][:, _GPERM].reshape(4, 128, G4))
    wih2t = np.empty((97, G4), f)
    wih2t[0:96] = Wih2.T[512:608][:, _GPERM]
    wih2t[96] = (bih2 + bhh2)[_GPERM]
    shared["wih2t"] = wih2t
    shared["whh2"] = np.ascontiguousarray(
        Whh2.T[:, _GPERM].reshape(4, 128, G4))
    shared["w1h"] = np.ascontiguousarray(W1.T[0:512].reshape(4, 128, H))
    shared["w1t"] = np.ascontiguousarray(W1.T[512:576])
    shared["b1"] = np.ascontiguousarray(b1.reshape(4, 128).T)
    shared["w2"] = np.ascontiguousarray(W2.T.reshape(4, 128, D))
    shared["b2"] = np.ascontiguousarray(b2.reshape(D, 1))

    state0 = actions[0]  # [B, A]
    in_maps = []
    for c in range(NCORES):
        bs = slice(c * BL, (c + 1) * BL)
        m = dict(shared)
        tacT = np.empty((65, CONTEXT, BL), f)
        for t in range(CONTEXT):
            tacT[0:64, t] = tactiles[t, bs].T
        tacT[64] = 1.0
        m["tacT"] = tacT
        tiledT = np.empty((NSTEPS_FULL, 97, BL), f)
        for t in range(NSTEPS_FULL):
            tiled = np.concatenate([actions[t + 1, bs], state0[bs]] * 4,
                                   axis=1)  # [BL, 96]
            tiledT[t, 0:96] = tiled.T
            tiledT[t, 96] = 1.0
        m["tiledT"] = tiledT
        m["targT"] = np.ascontiguousarray(
            tactiles[1:].transpose(0, 2, 1)[:, :, bs])
        in_maps.append(m)
    return in_maps


_CACHE = {}


def run(inputs, n_steps=NSTEPS_FULL, trace=True):
    key = n_steps
    if key not in _CACHE:
        _CACHE[key] = build(n_steps)
    nc = _CACHE[key]
    in_maps = prep_inputs(**inputs)
    res = run_bass_kernel_spmd(nc, in_maps, core_ids=list(range(NCORES)),
                               trace=trace)
    n_out = max(n_steps - (CONTEXT - 1), 0)
    outs = np.concatenate(
        [res.results[c]["outs"][:n_out].transpose(0, 2, 1)
         for c in range(NCORES)], axis=1)  # [n_out, B, D]
    mae_sum = sum(float(res.results[c]["mae"].sum()) for c in range(NCORES))
    mae = np.float32(mae_sum / (B * D) / (NSTEPS_FULL))
    return (mae, outs), res


def kernel(**inputs):
    (mae, outs), _ = run(inputs, trace=False)
    return mae, outs


# revision 4
# speedup vs baseline: 1.0981x; 1.0981x over previous
"""Trainium2 Bass kernel for the 2-layer-LSTM tactile rollout (nn_ACTP).

Strategy: data-parallel over batch (B=512 -> 64 rows/core on 8 cores).
All weights stay SBUF-resident (f32r, DMA'd directly). LSTM + FC1
matmuls run batch-major (activations stationary, weights moving,
float32r at N=512 -> full PE rate); FC2 runs feature-major (weights
stationary) so the rollout feedback path needs no extra transpose.
Gate order is reshuffled to [g i f o]; each gate gets its own [64,512]
PSUM bank so activations consume chunks as the PE closes them and the
next step's recurrence matmuls start early. Gate/FC1 biases ride a
ones-row in the stationary operand; the FC2 bias uses the ACT
per-partition bias port. h1/h2/out3 are re-transposed each step via
PE-transpose (4x [64,128] each). mae partials exit as a [64,1]
per-core column, reduced on host.
"""
import sys
import types

sys.path.insert(0, "/opt/trn_rl_repo")

import numpy as np

# ---- NTFF profile hook (missing antenv.axon_hooks in this image) ----
try:
    import antenv
    if "antenv.axon_hooks" not in sys.modules:
        _mod = types.ModuleType("antenv.axon_hooks")
        _hooks = [None]
        _mod.set_axon_ntff_profile_hook = lambda h: _hooks.__setitem__(0, h)
        _mod.get_axon_ntff_profile_hook = lambda: _hooks[0]
        sys.modules["antenv.axon_hooks"] = _mod
        antenv.axon_hooks = _mod
        from trn_agent_boot.trn_boot import _ntff_profile_via_ctypes
        _mod.set_axon_ntff_profile_hook(
            _ntff_profile_via_ctypes("/opt/axon/libaxon_pjrt.so")
        )
except Exception:
    pass

import concourse.bass as bass
import concourse.mybir as mybir
import concourse.tile as tile
from concourse import bacc
from concourse.bass_utils import run_bass_kernel_spmd
from concourse.masks import make_identity

F32 = mybir.dt.float32
F32R = mybir.dt.float32r

T, B, D, A, H = 101, 512, 64, 12, 512
CONTEXT = 10
NCORES = 8
BL = B // NCORES          # 64 batch rows per core
NSTEPS_FULL = T - 1       # 100
G4 = 4 * H                # 2048

# gate chunk order: 0=g (tanh), 1=i, 2=f, 3=o (sigmoid)
SIGF = mybir.ActivationFunctionType.Sigmoid
TANHF = mybir.ActivationFunctionType.Tanh


def build(n_steps=NSTEPS_FULL):
    nc = bacc.Bacc("TRN2", target_bir_lowering=False, debug=False,
                   num_devices=NCORES)

    dp = lambda name, shape, dt=F32: nc.dram_tensor(
        name, shape, dt, kind="ExternalInput").ap()

    d = {
        "wih1": dp("wih1", [65, G4]),
        "whh1": dp("whh1", [4, 128, G4]),
        "wih2h": dp("wih2h", [4, 128, G4]),
        "wih2t": dp("wih2t", [97, G4]),
        "whh2": dp("whh2", [4, 128, G4]),
        "w1h": dp("w1h", [4, 128, H]),
        "w1t": dp("w1t", [65, H]),
        "w2": dp("w2", [4, 128, D]),
        "b2": dp("b2", [64, 1]),
        "tacT": dp("tacT", [65, CONTEXT, BL]),
        "tiledT": dp("tiledT", [NSTEPS_FULL, 97, BL]),
        "targT": dp("targT", [NSTEPS_FULL, D, BL]),
    }
    n_out = max(n_steps - (CONTEXT - 1), 0)
    d["outs"] = nc.dram_tensor("outs", [max(n_out, 1), D, BL], F32,
                               kind="ExternalOutput").ap()
    d["mae"] = nc.dram_tensor("mae", [D, 1], F32,
                              kind="ExternalOutput").ap()

    with tile.TileContext(nc) as tc:
        _body(nc, tc, n_steps, d)
    nc.finalize()
    return nc


def _body(nc, tc, n_steps, d):
    from contextlib import ExitStack
    ctx = ExitStack()
    wp = ctx.enter_context(tc.tile_pool(name="wp", bufs=1))
    st = ctx.enter_context(tc.tile_pool(name="st", bufs=1))
    wk = ctx.enter_context(tc.tile_pool(name="wk", bufs=2))
    sm = ctx.enter_context(tc.tile_pool(name="sm", bufs=3))
    gps = ctx.enter_context(tc.tile_pool(name="gps", bufs=4, space="PSUM"))
    tps = ctx.enter_context(tc.tile_pool(name="tps", bufs=2, space="PSUM"))
    fps = ctx.enter_context(tc.tile_pool(name="fps", bufs=2, space="PSUM"))

    r = lambda ap: ap.bitcast(F32R)

    # ---- resident weights (f32r via direct DMA) ----
    wih1 = wp.tile([65, G4], F32R)
    nc.sync.dma_start(out=wih1, in_=r(d["wih1"]))
    whh1 = wp.tile([128, 4, G4], F32R)
    wih2h = wp.tile([128, 4, G4], F32R)
    whh2 = wp.tile([128, 4, G4], F32R)
    w1h = wp.tile([128, 4, H], F32R)
    w2 = wp.tile([128, 4, D], F32R)
    for k in range(4):
        nc.sync.dma_start(out=whh1[:, k, :], in_=r(d["whh1"][k]))
        nc.sync.dma_start(out=wih2h[:, k, :], in_=r(d["wih2h"][k]))
        nc.sync.dma_start(out=whh2[:, k, :], in_=r(d["whh2"][k]))
        nc.sync.dma_start(out=w1h[:, k, :], in_=r(d["w1h"][k]))
        nc.sync.dma_start(out=w2[:, k, :], in_=r(d["w2"][k]))
    wih2t = wp.tile([97, G4], F32R)
    nc.sync.dma_start(out=wih2t, in_=r(d["wih2t"]))
    w1t = wp.tile([65, H], F32R)
    nc.sync.dma_start(out=w1t, in_=r(d["w1t"]))
    b2 = wp.tile([64, 1], F32)
    nc.sync.dma_start(out=b2, in_=d["b2"])
    tacT = wp.tile([65, CONTEXT, BL], F32R)
    nc.sync.dma_start(out=tacT, in_=r(d["tacT"]))
    ident = wp.tile([128, 128], F32)
    make_identity(nc, ident[:])

    # ---- persistent state ----
    c1 = st.tile([BL, H], F32)
    c2 = st.tile([BL, H], F32)
    h1T = st.tile([128, 4, BL], F32R)
    h2T = st.tile([128, 4, BL], F32R)
    inp1T = st.tile([65, BL], F32R)
    out3T = st.tile([128, 4, BL], F32R)
    acc = st.tile([D, 1], F32)
    nc.vector.memset(inp1T[64:65, :].bitcast(F32), 1.0)

    def cell(chunks, cstate, h_out, first):
        """chunks = 4 psum tiles [BL,H]: 0=g 1=i 2=f 3=o."""
        tg = wk.tile([BL, H], F32, tag="tg")
        nc.scalar.activation(tg, chunks[0], TANHF)
        sig = wk.tile([BL, 3 * H], F32, tag="sig")
        nc.scalar.activation(sig[:, 0:H], chunks[1], SIGF)
        if first:
            nc.vector.tensor_mul(cstate[:], sig[:, 0:H], tg[:])
        else:
            nc.scalar.activation(sig[:, H:2 * H], chunks[2], SIGF)
            t2 = wk.tile([BL, H], F32, tag="t2")
            nc.vector.tensor_mul(t2, sig[:, 0:H], tg[:])
            t1 = wk.tile([BL, H], F32, tag="t1")
            nc.vector.tensor_mul(t1, sig[:, H:2 * H], cstate[:])
            nc.vector.tensor_add(cstate[:], t1[:], t2[:])
        nc.scalar.activation(sig[:, 2 * H:3 * H], chunks[3], SIGF)
        tcl = wk.tile([BL, H], F32, tag="tcl")
        nc.scalar.activation(tcl, cstate[:], TANHF)
        nc.vector.tensor_mul(h_out[:], sig[:, 2 * H:3 * H], tcl[:])

    def transpose_to(hT, h_bm, cast_dst=None):
        for k in range(4):
            tp = tps.tile([128, BL], F32, tag="tp")
            nc.tensor.transpose(tp, h_bm[:, 128 * k:128 * (k + 1)],
                                ident[0:BL, 0:BL])
            nc.vector.tensor_copy(hT[:, k, :], tp)

    NC = lambda n: slice(512 * n, 512 * (n + 1))
    g1c = None  # L1 gate-chunk psums carried across iterations
    for t in range(n_steps):
        teacher = t < CONTEXT
        inp_stat = tacT[:, t, :] if teacher else inp1T[:]

        # (1) L1 x-part (+bias row) closes each gate-chunk group
        if t == 0:
            g1c = [gps.tile([BL, H], F32, tag="gate", name=f"g1c{t}_{n}") for n in range(4)]
        for n in range(4):
            nc.tensor.matmul(g1c[n], inp_stat, wih1[:, NC(n)],
                             start=(t == 0), stop=True)

        # (2) L1 cell
        h1 = wk.tile([BL, H], F32, tag="h1")
        cell(g1c, c1, h1, first=(t == 0))

        # (3) L2 h-part opens L2 gate-chunk groups
        g2c = [gps.tile([BL, H], F32, tag="gate", name=f"g2c{t}_{n}") for n in range(4)]
        if t > 0:
            for n in range(4):
                for k in range(4):
                    nc.tensor.matmul(g2c[n], h2T[:, k, :],
                                     whh2[:, k, NC(n)],
                                     start=(k == 0), stop=False)

        # (4) h1 -> h1T
        transpose_to(h1T, h1)

        # (5) L2 x2-part: h1 contraction + tiled tail (bias row)
        tiledT = sm.tile([97, BL], F32R, tag="tiled")
        nc.sync.dma_start(out=tiledT, in_=r(d["tiledT"][t]))
        for n in range(4):
            for k in range(4):
                nc.tensor.matmul(g2c[n], h1T[:, k, :], wih2h[:, k, NC(n)],
                                 start=(t == 0 and k == 0), stop=False)
            nc.tensor.matmul(g2c[n], tiledT[:], wih2t[:, NC(n)],
                             start=False, stop=True)

        # (6) L2 cell
        h2 = wk.tile([BL, H], F32, tag="h2")
        cell(g2c, c2, h2, first=(t == 0))

        # (7) next step's L1 h-part opens the next gate-chunk groups early
        if t + 1 < n_steps:
            g1c = [gps.tile([BL, H], F32, tag="gate", name=f"g1c{t}_{n}") for n in range(4)]
            for n in range(4):
                for k in range(4):
                    nc.tensor.matmul(g1c[n], h1T[:, k, :],
                                     whh1[:, k, NC(n)],
                                     start=(k == 0), stop=False)

        # (8) h2 -> h2T
        transpose_to(h2T, h2)

        # (9) FC1 batch-major: out3 = tanh([h2, inp1, 1] @ [W1h; W1t|b1])
        fc1 = fps.tile([BL, H], F32, tag="fc")
        for k in range(4):
            nc.tensor.matmul(fc1, h2T[:, k, :], w1h[:, k, :],
                             start=(k == 0), stop=False)
        nc.tensor.matmul(fc1, inp_stat, w1t[:], start=False, stop=True)
        out3 = wk.tile([BL, H], F32, tag="out3")
        nc.scalar.activation(out3, fc1, TANHF)

        # (10) out3 -> out3T
        transpose_to(out3T, out3)

        # (11) FC2 feature-major: out4T = tanh(W2 @ out3T + b2)
        fc2 = fps.tile([D, BL], F32, tag="fc")
        for m in range(4):
            nc.tensor.matmul(fc2, w2[:, m, :], out3T[:, m, :],
                             start=(m == 0), stop=(m == 3))
        nc.scalar.activation(inp1T[0:D, :], fc2, TANHF, bias=b2[:, 0:1])

        if t >= CONTEXT - 1:
            nc.gpsimd.dma_start(out=d["outs"][t - (CONTEXT - 1)],
                                in_=inp1T[0:D, :].bitcast(F32))

        # (12) mae
        targT = sm.tile([D, BL], F32, tag="targ")
        nc.sync.dma_start(out=targT, in_=d["targT"][t])
        diff = wk.tile([D, BL], F32, tag="diff")
        nc.vector.tensor_sub(diff, inp1T[0:D, :].bitcast(F32), targT[:])
        if t == 0:
            nc.vector.tensor_reduce(acc[:], diff[:], mybir.AxisListType.X,
                                    mybir.AluOpType.add,
                                    apply_absolute_value=True)
        else:
            red = wk.tile([D, 1], F32, tag="red")
            nc.vector.tensor_reduce(red, diff[:], mybir.AxisListType.X,
                                    mybir.AluOpType.add,
                                    apply_absolute_value=True)
            nc.vector.tensor_add(acc[:], acc[:], red[:])

    nc.gpsimd.dma_start(out=d["mae"], in_=acc[:])
    ctx.close()


# ---------------- host side ----------------

# gate chunk order [g i f o] applied to the 4H gate axis (torch order i,f,g,o)
_GPERM = np.concatenate([np.arange(1024, 1536), np.arange(0, 512),
                         np.arange(512, 1024), np.arange(1536, 2048)])


def prep_inputs(tactiles, actions, Wih1, Whh1, bih1, bhh1, Wih2, Whh2,
                bih2, bhh2, W1, b1, W2, b2):
    f = np.float32
    shared = {}
    wih1 = np.empty((65, G4), f)
    wih1[0:64] = Wih1.T[:, _GPERM]
    wih1[64] = (bih1 + bhh1)[_GPERM]
    shared["wih1"] = wih1
    shared["whh1"] = np.ascontiguousarray(
        Whh1.T[:, _GPERM].reshape(4, 128, G4))
    shared["wih2h"] = np.ascontiguousarray(
        Wih2.T[0:512][:, _GPERM].reshape(4, 128, G4))
    wih2t = np.empty((97, G4), f)
    wih2t[0:96] = Wih2.T[512:608][:, _GPERM]
    wih2t[96] = (bih2 + bhh2)[_GPERM]
    shared["wih2t"] = wih2t
    shared["whh2"] = np.ascontiguousarray(
        Whh2.T[:, _GPERM].reshape(4, 128, G4))
    shared["w1h"] = np.ascontiguousarray(W1.T[0:512].reshape(4, 128, H))
    w1t = np.empty((65, H), f)
    w1t[0:64] = W1.T[512:576]
    w1t[64] = b1
    shared["w1t"] = w1t
    shared["w2"] = np.ascontiguousarray(W2.T.reshape(4, 128, D))
    shared["b2"] = np.ascontiguousarray(b2.reshape(D, 1))

    state0 = actions[0]  # [B, A]
    in_maps = []
    for c in range(NCORES):
        bs = slice(c * BL, (c + 1) * BL)
        m = dict(shared)
        tacT = np.empty((65, CONTEXT, BL), f)
        for t in range(CONTEXT):
            tacT[0:64, t] = tactiles[t, bs].T
        tacT[64] = 1.0
        m["tacT"] = tacT
        tiledT = np.empty((NSTEPS_FULL, 97, BL), f)
        for t in range(NSTEPS_FULL):
            tiled = np.concatenate([actions[t + 1, bs], state0[bs]] * 4,
                                   axis=1)  # [BL, 96]
            tiledT[t, 0:96] = tiled.T
            tiledT[t, 96] = 1.0
        m["tiledT"] = tiledT
        m["targT"] = np.ascontiguousarray(
            tactiles[1:].transpose(0, 2, 1)[:, :, bs])
        in_maps.append(m)
    return in_maps


_CACHE = {}


def run(inputs, n_steps=NSTEPS_FULL, trace=True):
    key = n_steps
    if key not in _CACHE:
        _CACHE[key] = build(n_steps)
    nc = _CACHE[key]
    in_maps = prep_inputs(**inputs)
    res = run_bass_kernel_spmd(nc, in_maps, core_ids=list(range(NCORES)),
                               trace=trace)
    n_out = max(n_steps - (CONTEXT - 1), 0)
    outs = np.concatenate(
        [res.results[c]["outs"][:n_out].transpose(0, 2, 1)
         for c in range(NCORES)], axis=1)  # [n_out, B, D]
    mae_sum = sum(float(res.results[c]["mae"].sum()) for c in range(NCORES))
    mae = np.float32(mae_sum / (B * D) / NSTEPS_FULL)
    return (mae, outs), res


def kernel(**inputs):
    (mae, outs), _ = run(inputs, trace=False)
    return mae, outs


# revision 5
# speedup vs baseline: 1.1989x; 1.0918x over previous
"""Trainium2 Bass kernel for the 2-layer-LSTM tactile rollout (nn_ACTP).

Strategy: data-parallel over batch (B=512 -> 64 rows/core on 8 cores).
All weights stay SBUF-resident (f32r, DMA'd directly). LSTM + FC1
matmuls run batch-major (activations stationary, weights moving,
float32r at N=512 -> full PE rate); FC2 runs feature-major (weights
stationary) so the rollout feedback path needs no extra transpose.
Gate order is reshuffled to [g i f o]; each gate gets its own [64,512]
PSUM bank so activations consume chunks as the PE closes them and the
next step's recurrence matmuls start early. Gate/FC1 biases ride a
ones-row in the stationary operand; the FC2 bias uses the ACT
per-partition bias port. h1/h2/out3 are re-transposed each step via
PE-transpose (4x [64,128] each). mae partials exit as a [64,1]
per-core column, reduced on host.
"""
import sys
import types

sys.path.insert(0, "/opt/trn_rl_repo")

import numpy as np

# ---- NTFF profile hook (missing antenv.axon_hooks in this image) ----
try:
    import antenv
    if "antenv.axon_hooks" not in sys.modules:
        _mod = types.ModuleType("antenv.axon_hooks")
        _hooks = [None]
        _mod.set_axon_ntff_profile_hook = lambda h: _hooks.__setitem__(0, h)
        _mod.get_axon_ntff_profile_hook = lambda: _hooks[0]
        sys.modules["antenv.axon_hooks"] = _mod
        antenv.axon_hooks = _mod
        from trn_agent_boot.trn_boot import _ntff_profile_via_ctypes
        _mod.set_axon_ntff_profile_hook(
            _ntff_profile_via_ctypes("/opt/axon/libaxon_pjrt.so")
        )
except Exception:
    pass

import concourse.bass as bass
import concourse.mybir as mybir
import concourse.tile as tile
from concourse import bacc
from concourse.bass_utils import run_bass_kernel_spmd
from concourse.masks import make_identity

F32 = mybir.dt.float32
F32R = mybir.dt.float32r
import ml_dtypes
MM_BF16 = True
MMDT = mybir.dt.bfloat16 if MM_BF16 else F32R
NPDT = ml_dtypes.bfloat16 if MM_BF16 else np.float32

T, B, D, A, H = 101, 512, 64, 12, 512
CONTEXT = 10
NCORES = 8
BL = B // NCORES          # 64 batch rows per core
NSTEPS_FULL = T - 1       # 100
G4 = 4 * H                # 2048

# gate chunk order: 0=g (tanh), 1=i, 2=f, 3=o (sigmoid)
SIGF = mybir.ActivationFunctionType.Sigmoid
TANHF = mybir.ActivationFunctionType.Tanh


def build(n_steps=NSTEPS_FULL):
    nc = bacc.Bacc("TRN2", target_bir_lowering=False, debug=False,
                   num_devices=NCORES)

    dp = lambda name, shape, dt=MMDT: nc.dram_tensor(
        name, shape, dt, kind="ExternalInput").ap()

    d = {
        "wih1": dp("wih1", [65, G4]),
        "whh1": dp("whh1", [4, 128, G4]),
        "wih2h": dp("wih2h", [4, 128, G4]),
        "wih2t": dp("wih2t", [97, G4]),
        "whh2": dp("whh2", [4, 128, G4]),
        "w1h": dp("w1h", [4, 128, H]),
        "w1t": dp("w1t", [65, H]),
        "w2": dp("w2", [4, 128, D]),
        "b2": dp("b2", [64, 1], F32),
        "tacT": dp("tacT", [65, CONTEXT, BL]),
        "tiledT": dp("tiledT", [NSTEPS_FULL, 97, BL]),
        "targT": dp("targT", [NSTEPS_FULL, D, BL], F32),
    }
    n_out = max(n_steps - (CONTEXT - 1), 0)
    d["outs"] = nc.dram_tensor("outs", [max(n_out, 1), D, BL], F32,
                               kind="ExternalOutput").ap()
    d["mae"] = nc.dram_tensor("mae", [D, 1], F32,
                              kind="ExternalOutput").ap()

    with tile.TileContext(nc) as tc:
        _body(nc, tc, n_steps, d)
    nc.finalize()
    return nc


def _body(nc, tc, n_steps, d):
    from contextlib import ExitStack
    ctx = ExitStack()
    wp = ctx.enter_context(tc.tile_pool(name="wp", bufs=1))
    st = ctx.enter_context(tc.tile_pool(name="st", bufs=1))
    wk = ctx.enter_context(tc.tile_pool(name="wk", bufs=2))
    sm = ctx.enter_context(tc.tile_pool(name="sm", bufs=3))
    gps = ctx.enter_context(tc.tile_pool(name="gps", bufs=4, space="PSUM"))
    tps = ctx.enter_context(tc.tile_pool(name="tps", bufs=2, space="PSUM"))
    fps = ctx.enter_context(tc.tile_pool(name="fps", bufs=2, space="PSUM"))

    r = lambda ap: ap.bitcast(MMDT) if not MM_BF16 else ap

    # ---- resident weights (f32r via direct DMA) ----
    wih1 = wp.tile([65, G4], MMDT)
    nc.sync.dma_start(out=wih1, in_=r(d["wih1"]))
    whh1 = wp.tile([128, 4, G4], MMDT)
    wih2h = wp.tile([128, 4, G4], MMDT)
    whh2 = wp.tile([128, 4, G4], MMDT)
    w1h = wp.tile([128, 4, H], MMDT)
    w2 = wp.tile([128, 4, D], MMDT)
    for k in range(4):
        nc.sync.dma_start(out=whh1[:, k, :], in_=r(d["whh1"][k]))
        nc.sync.dma_start(out=wih2h[:, k, :], in_=r(d["wih2h"][k]))
        nc.sync.dma_start(out=whh2[:, k, :], in_=r(d["whh2"][k]))
        nc.sync.dma_start(out=w1h[:, k, :], in_=r(d["w1h"][k]))
        nc.sync.dma_start(out=w2[:, k, :], in_=r(d["w2"][k]))
    wih2t = wp.tile([97, G4], MMDT)
    nc.sync.dma_start(out=wih2t, in_=r(d["wih2t"]))
    w1t = wp.tile([65, H], MMDT)
    nc.sync.dma_start(out=w1t, in_=r(d["w1t"]))
    b2 = wp.tile([64, 1], F32)
    nc.sync.dma_start(out=b2, in_=d["b2"])
    tacT = wp.tile([65, CONTEXT, BL], MMDT)
    nc.sync.dma_start(out=tacT, in_=r(d["tacT"]))
    ident = wp.tile([128, 128], F32)
    make_identity(nc, ident[:])

    # ---- persistent state ----
    c1 = st.tile([BL, H], F32)
    c2 = st.tile([BL, H], F32)
    h1T = st.tile([128, 4, BL], MMDT)
    h2T = st.tile([128, 4, BL], MMDT)
    inp1T = st.tile([65, BL], MMDT)
    out3T = st.tile([128, 4, BL], MMDT)
    acc = st.tile([D, 1], F32)
    nc.vector.memset(inp1T[64:65, :] if MM_BF16 else inp1T[64:65, :].bitcast(F32), 1.0)

    def cell(chunks, cstate, h_out, first):
        """chunks = 4 psum tiles [BL,H]: 0=g 1=i 2=f 3=o."""
        tg = wk.tile([BL, H], F32, tag="tg")
        nc.scalar.activation(tg, chunks[0], TANHF)
        sig = wk.tile([BL, 3 * H], F32, tag="sig")
        nc.scalar.activation(sig[:, 0:H], chunks[1], SIGF)
        if first:
            nc.vector.tensor_mul(cstate[:], sig[:, 0:H], tg[:])
        else:
            nc.scalar.activation(sig[:, H:2 * H], chunks[2], SIGF)
            t2 = wk.tile([BL, H], F32, tag="t2")
            nc.vector.tensor_mul(t2, sig[:, 0:H], tg[:])
            t1 = wk.tile([BL, H], F32, tag="t1")
            nc.vector.tensor_mul(t1, sig[:, H:2 * H], cstate[:])
            nc.vector.tensor_add(cstate[:], t1[:], t2[:])
        nc.scalar.activation(sig[:, 2 * H:3 * H], chunks[3], SIGF)
        tcl = wk.tile([BL, H], F32, tag="tcl")
        nc.scalar.activation(tcl, cstate[:], TANHF)
        nc.vector.tensor_mul(h_out[:], sig[:, 2 * H:3 * H], tcl[:])

    def transpose_to(hT, h_bm, cast_dst=None):
        for k in range(4):
            tp = tps.tile([128, BL], F32, tag="tp")
            nc.tensor.transpose(tp, h_bm[:, 128 * k:128 * (k + 1)],
                                ident[0:BL, 0:BL])
            nc.vector.tensor_copy(hT[:, k, :], tp)

    NC = lambda n: slice(512 * n, 512 * (n + 1))
    g1c = None  # L1 gate-chunk psums carried across iterations
    for t in range(n_steps):
        teacher = t < CONTEXT
        inp_stat = tacT[:, t, :] if teacher else inp1T[:]

        # (1) L1 x-part (+bias row) closes each gate-chunk group
        if t == 0:
            g1c = [gps.tile([BL, H], F32, tag="gate", name=f"g1c{t}_{n}") for n in range(4)]
        for n in range(4):
            nc.tensor.matmul(g1c[n], inp_stat, wih1[:, NC(n)],
                             start=(t == 0), stop=True)

        # (2) L1 cell
        h1 = wk.tile([BL, H], F32, tag="h1")
        cell(g1c, c1, h1, first=(t == 0))

        # (3) L2 h-part opens L2 gate-chunk groups
        g2c = [gps.tile([BL, H], F32, tag="gate", name=f"g2c{t}_{n}") for n in range(4)]
        if t > 0:
            for n in range(4):
                for k in range(4):
                    nc.tensor.matmul(g2c[n], h2T[:, k, :],
                                     whh2[:, k, NC(n)],
                                     start=(k == 0), stop=False)

        # (4) h1 -> h1T
        transpose_to(h1T, h1)

        # (5) L2 x2-part: h1 contraction + tiled tail (bias row)
        tiledT = sm.tile([97, BL], MMDT, tag="tiled")
        nc.sync.dma_start(out=tiledT, in_=r(d["tiledT"][t]))
        for n in range(4):
            for k in range(4):
                nc.tensor.matmul(g2c[n], h1T[:, k, :], wih2h[:, k, NC(n)],
                                 start=(t == 0 and k == 0), stop=False)
            nc.tensor.matmul(g2c[n], tiledT[:], wih2t[:, NC(n)],
                             start=False, stop=True)

        # (6) L2 cell
        h2 = wk.tile([BL, H], F32, tag="h2")
        cell(g2c, c2, h2, first=(t == 0))

        # (7) next step's L1 h-part opens the next gate-chunk groups early
        if t + 1 < n_steps:
            g1c = [gps.tile([BL, H], F32, tag="gate", name=f"g1c{t}_{n}") for n in range(4)]
            for n in range(4):
                for k in range(4):
                    nc.tensor.matmul(g1c[n], h1T[:, k, :],
                                     whh1[:, k, NC(n)],
                                     start=(k == 0), stop=False)

        # (8) h2 -> h2T
        transpose_to(h2T, h2)

        # (9) FC1 batch-major: out3 = tanh([h2, inp1, 1] @ [W1h; W1t|b1])
        fc1 = fps.tile([BL, H], F32, tag="fc")
        for k in range(4):
            nc.tensor.matmul(fc1, h2T[:, k, :], w1h[:, k, :],
                             start=(k == 0), stop=False)
        nc.tensor.matmul(fc1, inp_stat, w1t[:], start=False, stop=True)
        out3 = wk.tile([BL, H], F32, tag="out3")
        nc.scalar.activation(out3, fc1, TANHF)

        # (10) out3 -> out3T
        transpose_to(out3T, out3)

        # (11) FC2 feature-major: out4T = tanh(W2 @ out3T + b2)
        fc2 = fps.tile([D, BL], F32, tag="fc")
        for m in range(4):
            nc.tensor.matmul(fc2, w2[:, m, :], out3T[:, m, :],
                             start=(m == 0), stop=(m == 3))
        nc.scalar.activation(inp1T[0:D, :], fc2, TANHF, bias=b2[:, 0:1])

        out4f = wk.tile([D, BL], F32, tag="out4f")
        nc.scalar.activation(out4f, fc2, TANHF, bias=b2[:, 0:1])
        if t >= CONTEXT - 1:
            nc.gpsimd.dma_start(out=d["outs"][t - (CONTEXT - 1)], in_=out4f[:])

        # (12) mae
        targT = sm.tile([D, BL], F32, tag="targ")
        nc.sync.dma_start(out=targT, in_=d["targT"][t])
        diff = wk.tile([D, BL], F32, tag="diff")
        nc.vector.tensor_sub(diff, out4f[:], targT[:])
        if t == 0:
            nc.vector.tensor_reduce(acc[:], diff[:], mybir.AxisListType.X,
                                    mybir.AluOpType.add,
                                    apply_absolute_value=True)
        else:
            red = wk.tile([D, 1], F32, tag="red")
            nc.vector.tensor_reduce(red, diff[:], mybir.AxisListType.X,
                                    mybir.AluOpType.add,
                                    apply_absolute_value=True)
            nc.vector.tensor_add(acc[:], acc[:], red[:])

    nc.gpsimd.dma_start(out=d["mae"], in_=acc[:])
    ctx.close()


# ---------------- host side ----------------

# gate chunk order [g i f o] applied to the 4H gate axis (torch order i,f,g,o)
_GPERM = np.concatenate([np.arange(1024, 1536), np.arange(0, 512),
                         np.arange(512, 1024), np.arange(1536, 2048)])


def prep_inputs(tactiles, actions, Wih1, Whh1, bih1, bhh1, Wih2, Whh2,
                bih2, bhh2, W1, b1, W2, b2):
    f = np.float32
    shared = {}
    wih1 = np.empty((65, G4), f)
    wih1[0:64] = Wih1.T[:, _GPERM]
    wih1[64] = (bih1 + bhh1)[_GPERM]
    shared["wih1"] = wih1
    shared["whh1"] = np.ascontiguousarray(
        Whh1.T[:, _GPERM].reshape(4, 128, G4))
    shared["wih2h"] = np.ascontiguousarray(
        Wih2.T[0:512][:, _GPERM].reshape(4, 128, G4))
    wih2t = np.empty((97, G4), f)
    wih2t[0:96] = Wih2.T[512:608][:, _GPERM]
    wih2t[96] = (bih2 + bhh2)[_GPERM]
    shared["wih2t"] = wih2t
    shared["whh2"] = np.ascontiguousarray(
        Whh2.T[:, _GPERM].reshape(4, 128, G4))
    shared["w1h"] = np.ascontiguousarray(W1.T[0:512].reshape(4, 128, H))
    w1t = np.empty((65, H), f)
    w1t[0:64] = W1.T[512:576]
    w1t[64] = b1
    shared["w1t"] = w1t
    shared["w2"] = np.ascontiguousarray(W2.T.reshape(4, 128, D))
    shared["b2"] = np.ascontiguousarray(b2.reshape(D, 1))

    state0 = actions[0]  # [B, A]
    in_maps = []
    for c in range(NCORES):
        bs = slice(c * BL, (c + 1) * BL)
        m = dict(shared)
        tacT = np.empty((65, CONTEXT, BL), f)
        for t in range(CONTEXT):
            tacT[0:64, t] = tactiles[t, bs].T
        tacT[64] = 1.0
        m["tacT"] = tacT
        tiledT = np.empty((NSTEPS_FULL, 97, BL), f)
        for t in range(NSTEPS_FULL):
            tiled = np.concatenate([actions[t + 1, bs], state0[bs]] * 4,
                                   axis=1)  # [BL, 96]
            tiledT[t, 0:96] = tiled.T
            tiledT[t, 96] = 1.0
        m["tiledT"] = tiledT
        m["targT"] = np.ascontiguousarray(
            tactiles[1:].transpose(0, 2, 1)[:, :, bs])
        keep_f32 = {"b2", "targT"}
        m = {k: (v if k in keep_f32 else v.astype(NPDT)) for k, v in m.items()}
        in_maps.append(m)
    return in_maps


_CACHE = {}


def run(inputs, n_steps=NSTEPS_FULL, trace=True):
    key = n_steps
    if key not in _CACHE:
        _CACHE[key] = build(n_steps)
    nc = _CACHE[key]
    in_maps = prep_inputs(**inputs)
    res = run_bass_kernel_spmd(nc, in_maps, core_ids=list(range(NCORES)),
                               trace=trace)
    n_out = max(n_steps - (CONTEXT - 1), 0)
    outs = np.concatenate(
        [res.results[c]["outs"][:n_out].transpose(0, 2, 1)
         for c in range(NCORES)], axis=1)  # [n_out, B, D]
    mae_sum = sum(float(res.results[c]["mae"].sum()) for c in range(NCORES))
    mae = np.float32(mae_sum / (B * D) / NSTEPS_FULL)
    return (mae, outs), res


def kernel(**inputs):
    (mae, outs), _ = run(inputs, trace=False)
    return mae, outs
